# revision 10
# baseline (speedup 1.0000x reference)
"""DAT Swin block kernel for 8 Trainium2 NeuronCores.

Device (Bass/Tile, data-parallel over the 4096-window batch, weights
replicated): the four dense 256x256 projections (q, k, v, o) — the dominant
FLOPs and all large-tensor HBM traffic — run in bf16 (1 PE cycle/row vs 4
for fp32, half the DMA bytes). Three launches (q; k+v fused; o), each
DMA-bandwidth-bound; tile sizes give 2-4KB per-partition DMA lines and
matmuls are split into 512-col halves so no PSUM bank boundary is crossed.
Host (numpy): the data-dependent glue that is layout-hostile on device
(offset network with cross-channel LN, bilinear gathers, 64x16 softmax).
"""
import sys, json

sys.path.insert(0, '/opt/trn_rl_repo')
import numpy as np
import concourse.bass as bass
import concourse.mybir as mybir
from concourse.tile import TileContext
from concourse import bass_utils

N_HEAD, N_GROUP, STRIDE, OFF_FACTOR, LN_EPS = 8, 4, 2, 2.0, 1e-5
NCORES = 8
BF16 = mybir.dt.np(mybir.dt.bfloat16)

# ---------------------------------------------------------------- BIR fix:
# this walrus build allows at most ONE sync wait per instruction; split
# extras onto same-engine NoOps inserted just before the instruction.
def _split_waits(m):
    ctr = [0]
    for f in m["functions"]:
        for b in f["blocks"]:
            out = []
            for i in b["instructions"]:
                si = i.get("sync_info")
                waits = (si or {}).get("on_wait") or []
                if len(waits) > 1:
                    for w in waits[:-1]:
                        ctr[0] += 1
                        out.append({"name": f"nopw-{ctr[0]}", "opcode": "NoOp",
                                    "engine": i.get("engine"), "ins": [], "outs": [],
                                    "sync_info": {"on_update": [], "on_wait": [w]}})
                    si["on_wait"] = waits[-1:]
                out.append(i)
            b["instructions"] = out
    return m


def _patch(nc):
    orig = nc.to_json_bytes
    nc.to_json_bytes = lambda: json.dumps(_split_waits(json.loads(orig()))).encode()
    return nc


# ------------------------------------------------- device projection kernel
# out[M, N] = wT.T @ xT + b   (wT: (256, M) bf16, xT: (256, N) bf16, b: (M,))
_NC_CACHE = {}


def _build_proj(M, N, nchunk=1024, iobufs=3, psbufs=2):
    key = (M, N, nchunk, iobufs, psbufs)
    if key in _NC_CACHE:
        return _NC_CACHE[key]
    nc = bass.Bass()
    f32 = mybir.dt.float32
    bf16 = mybir.dt.bfloat16
    xT = nc.dram_tensor("xT", (256, N), bf16, kind="ExternalInput")
    wT = nc.dram_tensor("wT", (256, M), bf16, kind="ExternalInput")
    bb = nc.dram_tensor("b", (M, 1), f32, kind="ExternalInput")
    out = nc.dram_tensor("out", (M, N), bf16, kind="ExternalOutput")
    n_mt = M // 128
    with TileContext(nc) as tc:
        with tc.tile_pool(name="w", bufs=1) as wp, \
             tc.tile_pool(name="io", bufs=iobufs) as iop, \
             tc.tile_pool(name="ps", bufs=psbufs, space="PSUM") as pp:
            wtiles = {}
            btiles = {}
            for mt in range(n_mt):
                for kt in range(2):
                    t = wp.tile([128, 128], bf16, tag=f"w{mt}{kt}")
                    nc.sync.dma_start(t[:, :], wT[kt * 128:(kt + 1) * 128,
                                                   mt * 128:(mt + 1) * 128])
                    wtiles[mt, kt] = t
                bt = wp.tile([128, 1], f32, tag=f"b{mt}")
                nc.sync.dma_start(bt[:, :], bb[mt * 128:(mt + 1) * 128, :])
                btiles[mt] = bt
            for n0 in range(0, N, nchunk):
                rhs = []
                for kt in range(2):
                    t = iop.tile([128, nchunk], bf16, tag=f"x{kt}")
                    nc.sync.dma_start(t[:, :], xT[kt * 128:(kt + 1) * 128,
                                                   n0:n0 + nchunk])
                    rhs.append(t)
                for mt in range(n_mt):
                    ot = iop.tile([128, nchunk], bf16, tag=f"o{mt % 2}")
                    # PSUM bank is 512 fp32 wide; matmul output cannot cross
                    # a bank boundary, so compute in 512-wide halves.
                    for h0 in range(0, nchunk, 512):
                        ps = pp.tile([128, 512], f32, tag="ps")
                        for kt in range(2):
                            nc.tensor.matmul(ps[:, :], wtiles[mt, kt][:, :],
                                             rhs[kt][:, h0:h0 + 512],
                                             start=(kt == 0), stop=(kt == 1))
                        nc.any.tensor_scalar_add(ot[:, h0:h0 + 512], ps[:, :],
                                                 btiles[mt][:, :])
                    nc.sync.dma_start(out[mt * 128:(mt + 1) * 128,
                                          n0:n0 + nchunk], ot[:, :])
    _NC_CACHE[key] = _patch(nc)
    return _NC_CACHE[key]


_EXEC_NS = [0]


def _run_proj(xT_sh, wT, b):
    """xT_sh: list of NCORES bf16 arrays (256, N). Returns list of (M, N)."""
    M = wT.shape[1]
    N = xT_sh[0].shape[1]
    # pool sizes tuned via CoreSim sweep (sweep.py)
    if M == 512:
        nc = _build_proj(M, N, 1024, 6, 3)
    else:
        nc = _build_proj(M, N, 2048, 3, 6)
    wTb = np.ascontiguousarray(wT.astype(BF16))
    ins = [{"xT": np.ascontiguousarray(s), "wT": wTb,
            "b": np.ascontiguousarray(b.reshape(M, 1).astype(np.float32))}
           for s in xT_sh]
    res = bass_utils.run_bass_kernel_spmd(nc, ins, core_ids=list(range(NCORES)))
    if res.exec_time_ns:
        _EXEC_NS[0] += res.exec_time_ns
    return [r["out"] for r in res.results]


# ----------------------------------------------------------- host-side glue
def _grid_sample(img, grid):
    B, C, Hi, Wi = img.shape
    _, Hg, Wg, _ = grid.shape
    gx = (grid[..., 0] + 1.0) * 0.5 * (Wi - 1)
    gy = (grid[..., 1] + 1.0) * 0.5 * (Hi - 1)
    x0 = np.floor(gx); y0 = np.floor(gy)
    wx1 = gx - x0; wy1 = gy - y0
    flat = img.reshape(B, C, Hi * Wi)
    def corner(ix, iy, w):
        valid = (ix >= 0) & (ix <= Wi - 1) & (iy >= 0) & (iy <= Hi - 1)
        lin = (np.clip(iy, 0, Hi - 1) * Wi + np.clip(ix, 0, Wi - 1)).astype(np.int64)
        idx = np.broadcast_to(lin.reshape(B, 1, Hg * Wg), (B, C, Hg * Wg))
        v = np.take_along_axis(flat, idx, axis=2).reshape(B, C, Hg, Wg)
        return v * (w * valid)[:, None]
    return (corner(x0, y0, (1 - wx1) * (1 - wy1))
            + corner(x0 + 1, y0, wx1 * (1 - wy1))
            + corner(x0, y0 + 1, (1 - wx1) * wy1)
            + corner(x0 + 1, y0 + 1, wx1 * wy1))


def _ref_points(Hk, Wk, n):
    ry = ((np.arange(Hk, dtype=np.float32) + 0.5) / Hk) * 2 - 1
    rx = ((np.arange(Wk, dtype=np.float32) + 0.5) / Wk) * 2 - 1
    ref = np.stack(np.meshgrid(ry, rx, indexing='ij'), axis=-1)
    return np.broadcast_to(ref[None], (n, Hk, Wk, 2))


def kernel(x, wq, bq, wk, bk, wv, bv, wo, bo, off_dw_w, off_dw_b,
           off_ln_w, off_ln_b, off_pw_w, rpe_table, window_size):
    x = np.asarray(x, np.float32)
    ws = int(window_size)
    B, N, C = x.shape
    H = W = ws
    hc, gc, gh = C // N_HEAD, C // N_GROUP, N_HEAD // N_GROUP
    Bs = B // NCORES
    wq, wk, wv, wo = [np.asarray(a, np.float32) for a in (wq, wk, wv, wo)]

    # ---- q projection on device: q[d, (w,m)] = wq @ x^T + bq
    xT = np.ascontiguousarray(
        x.reshape(B, N, C).transpose(2, 0, 1).reshape(C, B * N).astype(BF16))
    shards = [np.ascontiguousarray(xT[:, i * Bs * N:(i + 1) * Bs * N])
              for i in range(NCORES)]
    qs = _run_proj(shards, wq.T, np.asarray(bq))
    q = np.concatenate(qs, axis=1).astype(np.float32)    # (C, B*N)
    q = q.reshape(C, B, N).transpose(1, 0, 2).reshape(B, C, H, W)

    # ---- offset network (host)
    q_off = q.reshape(B * N_GROUP, gc, H, W)
    Hk = Wk = (H + 2 - 3) // STRIDE + 1
    t = np.zeros((B * N_GROUP, gc, Hk, Wk), np.float32)
    qp = np.pad(q_off, ((0, 0), (0, 0), (1, 1), (1, 1)))
    dw = np.asarray(off_dw_w, np.float32)
    for ky in range(3):
        for kx in range(3):
            t += dw[None, :, 0, ky, kx, None, None] * \
                 qp[:, :, ky:ky + 2 * Hk:2, kx:kx + 2 * Wk:2]
    t += np.asarray(off_dw_b, np.float32)[None, :, None, None]
    tt = t.transpose(0, 2, 3, 1)
    mu = tt.mean(-1, keepdims=True)
    var = ((tt - mu) ** 2).mean(-1, keepdims=True)
    tt = (tt - mu) / np.sqrt(var + LN_EPS) * off_ln_w + off_ln_b
    from scipy.special import erf  # exact gelu
    tt = tt * 0.5 * (1.0 + erf(tt / np.sqrt(2.0)))
    offset = (tt.reshape(-1, gc) @ np.asarray(off_pw_w, np.float32).T) \
        .reshape(tt.shape[:3] + (2,))
    orange = np.array([1.0 / (Hk - 1), 1.0 / (Wk - 1)], np.float32)
    offset = np.tanh(offset) * orange * OFF_FACTOR
    n_sample = Hk * Wk
    pos = offset + _ref_points(Hk, Wk, B * N_GROUP)

    # ---- deformable gather (host) then k/v projections (device)
    xi = x.reshape(B, H, W, C).transpose(0, 3, 1, 2)
    xs = _grid_sample(xi.reshape(B * N_GROUP, gc, H, W), pos[..., ::-1])
    xs = xs.reshape(B, C, n_sample)
    xsT = np.ascontiguousarray(
        xs.transpose(1, 0, 2).reshape(C, B * n_sample).astype(BF16))
    shards = [np.ascontiguousarray(xsT[:, i * Bs * n_sample:(i + 1) * Bs * n_sample])
              for i in range(NCORES)]
    wkv = np.concatenate([wk.T, wv.T], axis=1)          # (256, 512)
    bkv = np.concatenate([np.asarray(bk), np.asarray(bv)])
    kvs = _run_proj(shards, wkv, bkv)
    kv = np.concatenate(kvs, axis=1).astype(np.float32)
    kv = kv.reshape(2 * C, B, n_sample).transpose(1, 0, 2)
    k, v = kv[:, :C], kv[:, C:]

    # ---- attention + rpe bias + softmax (host)
    qh = q.reshape(B * N_HEAD, hc, H * W)
    kh = k.reshape(B * N_HEAD, hc, n_sample)
    vh = v.reshape(B * N_HEAD, hc, n_sample)
    attn = np.matmul(qh.transpose(0, 2, 1), kh) * (hc ** -0.5)
    rpe = np.broadcast_to(np.asarray(rpe_table, np.float32)[None],
                          (B,) + rpe_table.shape)
    rpe = rpe.reshape(B * N_GROUP, gh, 2 * H - 1, 2 * W - 1)
    q_grid = _ref_points(H, W, B * N_GROUP).reshape(B * N_GROUP, H * W, 2)
    disp = (q_grid[:, :, None, :]
            - pos.reshape(B * N_GROUP, n_sample, 2)[:, None, :, :]) * 0.5
    bias = _grid_sample(rpe, disp[..., ::-1]).reshape(B * N_HEAD, H * W, n_sample)
    attn = attn + bias
    attn = attn - attn.max(axis=2, keepdims=True)
    e = np.exp(attn)
    attn = e / e.sum(axis=2, keepdims=True)
    out = np.matmul(vh, attn.transpose(0, 2, 1)).reshape(B, C, H * W)

    # ---- output projection on device
    outT = np.ascontiguousarray(
        out.transpose(1, 0, 2).reshape(C, B * N).astype(BF16))
    shards = [np.ascontiguousarray(outT[:, i * Bs * N:(i + 1) * Bs * N])
              for i in range(NCORES)]
    ys = _run_proj(shards, wo.T, np.asarray(bo))
    y = np.concatenate(ys, axis=1).astype(np.float32)
    y = y.reshape(C, B, N).transpose(1, 0, 2)
    return np.ascontiguousarray(y.reshape(B, C, H, W), dtype=np.float32)


# revision 13
# speedup vs baseline: 1.0235x; 1.0235x over previous
"""DAT Swin block kernel for 8 Trainium2 NeuronCores.

Device (Bass/Tile, data-parallel over the 4096-window batch, weights
replicated): the four dense 256x256 projections (q, k, v, o) — the dominant
FLOPs and all large-tensor HBM traffic — run in bf16 (1 PE cycle/row vs 4
for fp32, half the DMA bytes). Three launches (q; k+v fused; o), each
DMA-bandwidth-bound; tile sizes give 2-4KB per-partition DMA lines and
matmuls are split into 512-col halves so no PSUM bank boundary is crossed.
Host (numpy): the data-dependent glue that is layout-hostile on device
(offset network with cross-channel LN, bilinear gathers, 64x16 softmax).
"""
import sys, json

sys.path.insert(0, '/opt/trn_rl_repo')
import numpy as np
import concourse.bass as bass
import concourse.mybir as mybir
from concourse.tile import TileContext
from concourse import bass_utils

N_HEAD, N_GROUP, STRIDE, OFF_FACTOR, LN_EPS = 8, 4, 2, 2.0, 1e-5
NCORES = 8
BF16 = mybir.dt.np(mybir.dt.bfloat16)

# ---------------------------------------------------------------- BIR fix:
# this walrus build allows at most ONE sync wait per instruction; split
# extras onto same-engine NoOps inserted just before the instruction.
def _split_waits(m):
    ctr = [0]
    for f in m["functions"]:
        for b in f["blocks"]:
            out = []
            for i in b["instructions"]:
                si = i.get("sync_info")
                waits = (si or {}).get("on_wait") or []
                if len(waits) > 1:
                    for w in waits[:-1]:
                        ctr[0] += 1
                        out.append({"name": f"nopw-{ctr[0]}", "opcode": "NoOp",
                                    "engine": i.get("engine"), "ins": [], "outs": [],
                                    "sync_info": {"on_update": [], "on_wait": [w]}})
                    si["on_wait"] = waits[-1:]
                out.append(i)
            b["instructions"] = out
    return m


def _patch(nc):
    orig = nc.to_json_bytes
    nc.to_json_bytes = lambda: json.dumps(_split_waits(json.loads(orig()))).encode()
    return nc


# ------------------------------------------------- device projection kernel
# out[M, N] = wT.T @ xT + b   (wT: (256, M) bf16, xT: (256, N) bf16, b: (M,))
_NC_CACHE = {}


def _build_proj(M, N, nchunk=1024, iobufs=3, psbufs=2):
    key = (M, N, nchunk, iobufs, psbufs)
    if key in _NC_CACHE:
        return _NC_CACHE[key]
    nc = bass.Bass()
    f32 = mybir.dt.float32
    bf16 = mybir.dt.bfloat16
    xT = nc.dram_tensor("xT", (256, N), bf16, kind="ExternalInput")
    wT = nc.dram_tensor("wT", (256, M), bf16, kind="ExternalInput")
    bb = nc.dram_tensor("b", (M, 1), f32, kind="ExternalInput")
    out = nc.dram_tensor("out", (M, N), bf16, kind="ExternalOutput")
    n_mt = M // 128
    with TileContext(nc) as tc:
        with tc.tile_pool(name="w", bufs=1) as wp, \
             tc.tile_pool(name="io", bufs=iobufs) as iop, \
             tc.tile_pool(name="ps", bufs=psbufs, space="PSUM") as pp:
            wtiles = {}
            btiles = {}
            for kt in range(2):
                t = wp.tile([128, M], bf16, tag=f"w{kt}")
                nc.sync.dma_start(t[:, :], wT[kt * 128:(kt + 1) * 128, :])
                wtiles[kt] = t
            for mt in range(n_mt):
                bt = wp.tile([128, 1], f32, tag=f"b{mt}")
                nc.sync.dma_start(bt[:, :], bb[mt * 128:(mt + 1) * 128, :])
                btiles[mt] = bt
            for n0 in range(0, N, nchunk):
                rhs = []
                for kt in range(2):
                    t = iop.tile([128, nchunk], bf16, tag=f"x{kt}")
                    nc.sync.dma_start(t[:, :], xT[kt * 128:(kt + 1) * 128,
                                                   n0:n0 + nchunk])
                    rhs.append(t)
                for mt in range(n_mt):
                    ot = iop.tile([128, nchunk], bf16, tag=f"o{mt % 2}")
                    # PSUM bank is 512 fp32 wide; matmul output cannot cross
                    # a bank boundary, so compute in 512-wide halves.
                    for h0 in range(0, nchunk, 512):
                        ps = pp.tile([128, 512], f32, tag="ps")
                        for kt in range(2):
                            nc.tensor.matmul(ps[:, :],
                                             wtiles[kt][:, mt * 128:(mt + 1) * 128],
                                             rhs[kt][:, h0:h0 + 512],
                                             start=(kt == 0), stop=(kt == 1))
                        nc.any.tensor_scalar_add(ot[:, h0:h0 + 512], ps[:, :],
                                                 btiles[mt][:, :])
                    nc.sync.dma_start(out[mt * 128:(mt + 1) * 128,
                                          n0:n0 + nchunk], ot[:, :])
    _NC_CACHE[key] = _patch(nc)
    return _NC_CACHE[key]


_EXEC_NS = [0]


def _run_proj(xT_sh, wT, b):
    """xT_sh: list of NCORES bf16 arrays (256, N). Returns list of (M, N)."""
    M = wT.shape[1]
    N = xT_sh[0].shape[1]
    # pool sizes tuned via CoreSim sweep (sweep.py)
    if M == 512:
        nc = _build_proj(M, N, 1024, 6, 3)
    else:
        nc = _build_proj(M, N, 4096, 3, 6)
    wTb = np.ascontiguousarray(wT.astype(BF16))
    ins = [{"xT": np.ascontiguousarray(s), "wT": wTb,
            "b": np.ascontiguousarray(b.reshape(M, 1).astype(np.float32))}
           for s in xT_sh]
    res = bass_utils.run_bass_kernel_spmd(nc, ins, core_ids=list(range(NCORES)))
    if res.exec_time_ns:
        _EXEC_NS[0] += res.exec_time_ns
    return [r["out"] for r in res.results]


# ----------------------------------------------------------- host-side glue
def _grid_sample(img, grid):
    B, C, Hi, Wi = img.shape
    _, Hg, Wg, _ = grid.shape
    gx = (grid[..., 0] + 1.0) * 0.5 * (Wi - 1)
    gy = (grid[..., 1] + 1.0) * 0.5 * (Hi - 1)
    x0 = np.floor(gx); y0 = np.floor(gy)
    wx1 = gx - x0; wy1 = gy - y0
    flat = img.reshape(B, C, Hi * Wi)
    def corner(ix, iy, w):
        valid = (ix >= 0) & (ix <= Wi - 1) & (iy >= 0) & (iy <= Hi - 1)
        lin = (np.clip(iy, 0, Hi - 1) * Wi + np.clip(ix, 0, Wi - 1)).astype(np.int64)
        idx = np.broadcast_to(lin.reshape(B, 1, Hg * Wg), (B, C, Hg * Wg))
        v = np.take_along_axis(flat, idx, axis=2).reshape(B, C, Hg, Wg)
        return v * (w * valid)[:, None]
    return (corner(x0, y0, (1 - wx1) * (1 - wy1))
            + corner(x0 + 1, y0, wx1 * (1 - wy1))
            + corner(x0, y0 + 1, (1 - wx1) * wy1)
            + corner(x0 + 1, y0 + 1, wx1 * wy1))


def _ref_points(Hk, Wk, n):
    ry = ((np.arange(Hk, dtype=np.float32) + 0.5) / Hk) * 2 - 1
    rx = ((np.arange(Wk, dtype=np.float32) + 0.5) / Wk) * 2 - 1
    ref = np.stack(np.meshgrid(ry, rx, indexing='ij'), axis=-1)
    return np.broadcast_to(ref[None], (n, Hk, Wk, 2))


def kernel(x, wq, bq, wk, bk, wv, bv, wo, bo, off_dw_w, off_dw_b,
           off_ln_w, off_ln_b, off_pw_w, rpe_table, window_size):
    x = np.asarray(x, np.float32)
    ws = int(window_size)
    B, N, C = x.shape
    H = W = ws
    hc, gc, gh = C // N_HEAD, C // N_GROUP, N_HEAD // N_GROUP
    Bs = B // NCORES
    wq, wk, wv, wo = [np.asarray(a, np.float32) for a in (wq, wk, wv, wo)]

    # ---- q projection on device: q[d, (w,m)] = wq @ x^T + bq
    xT = np.ascontiguousarray(
        x.reshape(B, N, C).transpose(2, 0, 1).reshape(C, B * N).astype(BF16))
    shards = [np.ascontiguousarray(xT[:, i * Bs * N:(i + 1) * Bs * N])
              for i in range(NCORES)]
    qs = _run_proj(shards, wq.T, np.asarray(bq))
    q = np.concatenate(qs, axis=1).astype(np.float32)    # (C, B*N)
    q = q.reshape(C, B, N).transpose(1, 0, 2).reshape(B, C, H, W)

    # ---- offset network (host)
    q_off = q.reshape(B * N_GROUP, gc, H, W)
    Hk = Wk = (H + 2 - 3) // STRIDE + 1
    t = np.zeros((B * N_GROUP, gc, Hk, Wk), np.float32)
    qp = np.pad(q_off, ((0, 0), (0, 0), (1, 1), (1, 1)))
    dw = np.asarray(off_dw_w, np.float32)
    for ky in range(3):
        for kx in range(3):
            t += dw[None, :, 0, ky, kx, None, None] * \
                 qp[:, :, ky:ky + 2 * Hk:2, kx:kx + 2 * Wk:2]
    t += np.asarray(off_dw_b, np.float32)[None, :, None, None]
    tt = t.transpose(0, 2, 3, 1)
    mu = tt.mean(-1, keepdims=True)
    var = ((tt - mu) ** 2).mean(-1, keepdims=True)
    tt = (tt - mu) / np.sqrt(var + LN_EPS) * off_ln_w + off_ln_b
    from scipy.special import erf  # exact gelu
    tt = tt * 0.5 * (1.0 + erf(tt / np.sqrt(2.0)))
    offset = (tt.reshape(-1, gc) @ np.asarray(off_pw_w, np.float32).T) \
        .reshape(tt.shape[:3] + (2,))
    orange = np.array([1.0 / (Hk - 1), 1.0 / (Wk - 1)], np.float32)
    offset = np.tanh(offset) * orange * OFF_FACTOR
    n_sample = Hk * Wk
    pos = offset + _ref_points(Hk, Wk, B * N_GROUP)

    # ---- deformable gather (host) then k/v projections (device)
    xi = x.reshape(B, H, W, C).transpose(0, 3, 1, 2)
    xs = _grid_sample(xi.reshape(B * N_GROUP, gc, H, W), pos[..., ::-1])
    xs = xs.reshape(B, C, n_sample)
    xsT = np.ascontiguousarray(
        xs.transpose(1, 0, 2).reshape(C, B * n_sample).astype(BF16))
    shards = [np.ascontiguousarray(xsT[:, i * Bs * n_sample:(i + 1) * Bs * n_sample])
              for i in range(NCORES)]
    wkv = np.concatenate([wk.T, wv.T], axis=1)          # (256, 512)
    bkv = np.concatenate([np.asarray(bk), np.asarray(bv)])
    kvs = _run_proj(shards, wkv, bkv)
    kv = np.concatenate(kvs, axis=1).astype(np.float32)
    kv = kv.reshape(2 * C, B, n_sample).transpose(1, 0, 2)
    k, v = kv[:, :C], kv[:, C:]

    # ---- attention + rpe bias + softmax (host)
    qh = q.reshape(B * N_HEAD, hc, H * W)
    kh = k.reshape(B * N_HEAD, hc, n_sample)
    vh = v.reshape(B * N_HEAD, hc, n_sample)
    attn = np.matmul(qh.transpose(0, 2, 1), kh) * (hc ** -0.5)
    rpe = np.broadcast_to(np.asarray(rpe_table, np.float32)[None],
                          (B,) + rpe_table.shape)
    rpe = rpe.reshape(B * N_GROUP, gh, 2 * H - 1, 2 * W - 1)
    q_grid = _ref_points(H, W, B * N_GROUP).reshape(B * N_GROUP, H * W, 2)
    disp = (q_grid[:, :, None, :]
            - pos.reshape(B * N_GROUP, n_sample, 2)[:, None, :, :]) * 0.5
    bias = _grid_sample(rpe, disp[..., ::-1]).reshape(B * N_HEAD, H * W, n_sample)
    attn = attn + bias
    attn = attn - attn.max(axis=2, keepdims=True)
    e = np.exp(attn)
    attn = e / e.sum(axis=2, keepdims=True)
    out = np.matmul(vh, attn.transpose(0, 2, 1)).reshape(B, C, H * W)

    # ---- output projection on device
    outT = np.ascontiguousarray(
        out.transpose(1, 0, 2).reshape(C, B * N).astype(BF16))
    shards = [np.ascontiguousarray(outT[:, i * Bs * N:(i + 1) * Bs * N])
              for i in range(NCORES)]
    ys = _run_proj(shards, wo.T, np.asarray(bo))
    y = np.concatenate(ys, axis=1).astype(np.float32)
    y = y.reshape(C, B, N).transpose(1, 0, 2)
    return np.ascontiguousarray(y.reshape(B, C, H, W), dtype=np.float32)


# revision 17
# speedup vs baseline: 1.3449x; 1.3140x over previous
"""DAT Swin block kernel for 8 Trainium2 NeuronCores.

Device (Bass/Tile, data-parallel over the 4096-window batch, weights
replicated): the four dense 256x256 projections (q, k, v, o) — the dominant
FLOPs and all large-tensor HBM traffic — run in bf16 (1 PE cycle/row vs 4
for fp32, half the DMA bytes). Three launches (q; k+v fused; o), each
DMA-bandwidth-bound; tile sizes give 2-4KB per-partition DMA lines and
matmuls are split into 512-col halves so no PSUM bank boundary is crossed.
Host (numpy): the data-dependent glue that is layout-hostile on device
(offset network with cross-channel LN, bilinear gathers, 64x16 softmax).
"""
import sys, json

sys.path.insert(0, '/opt/trn_rl_repo')
import numpy as np
import concourse.bass as bass
import concourse.mybir as mybir
from concourse.tile import TileContext
from concourse import bass_utils

N_HEAD, N_GROUP, STRIDE, OFF_FACTOR, LN_EPS = 8, 4, 2, 2.0, 1e-5
NCORES = 8
BF16 = mybir.dt.np(mybir.dt.bfloat16)

# ---------------------------------------------------------------- BIR fix:
# this walrus build allows at most ONE sync wait per instruction; split
# extras onto same-engine NoOps inserted just before the instruction.
def _split_waits(m):
    ctr = [0]
    for f in m["functions"]:
        for b in f["blocks"]:
            out = []
            for i in b["instructions"]:
                si = i.get("sync_info")
                waits = (si or {}).get("on_wait") or []
                if len(waits) > 1:
                    for w in waits[:-1]:
                        ctr[0] += 1
                        out.append({"name": f"nopw-{ctr[0]}", "opcode": "NoOp",
                                    "engine": i.get("engine"), "ins": [], "outs": [],
                                    "sync_info": {"on_update": [], "on_wait": [w]}})
                    si["on_wait"] = waits[-1:]
                out.append(i)
            b["instructions"] = out
    return m


def _patch(nc):
    orig = nc.to_json_bytes
    nc.to_json_bytes = lambda: json.dumps(_split_waits(json.loads(orig()))).encode()
    return nc


# ------------------------------------------------- device projection kernel
# out[M, N] = wT.T @ xT + b   (wT: (256, M) bf16, xT: (256, N) bf16, b: (M,))
_NC_CACHE = {}


def _build_proj(M, N, nchunk=1024, iobufs=3, psbufs=2, outmod=2):
    """outmod: store-DMA issue pattern — index % outmod != outmod-1 goes to
    the ACT sequencer, the rest to SP. Splitting store issue off the SP
    queue keeps dependency-blocked stores from stalling input prefetch."""
    key = (M, N, nchunk, iobufs, psbufs, outmod)
    if key in _NC_CACHE:
        return _NC_CACHE[key]
    nc = bass.Bass()
    f32 = mybir.dt.float32
    bf16 = mybir.dt.bfloat16
    xT = nc.dram_tensor("xT", (256, N), bf16, kind="ExternalInput")
    wT = nc.dram_tensor("wT", (256, M), bf16, kind="ExternalInput")
    bb = nc.dram_tensor("b", (M, 1), f32, kind="ExternalInput")
    out = nc.dram_tensor("out", (M, N), bf16, kind="ExternalOutput")
    n_mt = M // 128
    with TileContext(nc) as tc:
        with tc.tile_pool(name="w", bufs=1) as wp, \
             tc.tile_pool(name="io", bufs=iobufs) as iop, \
             tc.tile_pool(name="ps", bufs=psbufs, space="PSUM") as pp:
            wtiles = {}
            btiles = {}
            for kt in range(2):
                t = wp.tile([128, M], bf16, tag=f"w{kt}")
                nc.sync.dma_start(t[:, :], wT[kt * 128:(kt + 1) * 128, :])
                wtiles[kt] = t
            for mt in range(n_mt):
                bt = wp.tile([128, 1], f32, tag=f"b{mt}")
                nc.sync.dma_start(bt[:, :], bb[mt * 128:(mt + 1) * 128, :])
                btiles[mt] = bt
            for ci, n0 in enumerate(range(0, N, nchunk)):
                rhs = []
                for kt in range(2):
                    t = iop.tile([128, nchunk], bf16, tag=f"x{kt}")
                    nc.sync.dma_start(t[:, :], xT[kt * 128:(kt + 1) * 128,
                                                   n0:n0 + nchunk])
                    rhs.append(t)
                for mt in range(n_mt):
                    ot = iop.tile([128, nchunk], bf16, tag=f"o{mt % 2}")
                    # PSUM bank is 512 fp32 wide; matmul output cannot cross
                    # a bank boundary, so compute in 512-wide halves.
                    for h0 in range(0, nchunk, 512):
                        ps = pp.tile([128, 512], f32, tag="ps")
                        for kt in range(2):
                            nc.tensor.matmul(ps[:, :],
                                             wtiles[kt][:, mt * 128:(mt + 1) * 128],
                                             rhs[kt][:, h0:h0 + 512],
                                             start=(kt == 0), stop=(kt == 1))
                        nc.any.tensor_scalar_add(ot[:, h0:h0 + 512], ps[:, :],
                                                 btiles[mt][:, :])
                    idx = ci * n_mt + mt
                    oeng = nc.scalar if idx % outmod != outmod - 1 else nc.sync
                    oeng.dma_start(out[mt * 128:(mt + 1) * 128,
                                       n0:n0 + nchunk], ot[:, :])
    _NC_CACHE[key] = _patch(nc)
    return _NC_CACHE[key]


_EXEC_NS = [0]


def _run_proj(xT_sh, wT, b):
    """xT_sh: list of NCORES bf16 arrays (256, N). Returns list of (M, N)."""
    M = wT.shape[1]
    N = xT_sh[0].shape[1]
    # pool sizes and store-issue pattern tuned via CoreSim sweep (sweep.py)
    if M == 512:
        nc = _build_proj(M, N, 1024, 6, 4, outmod=2)
    else:
        nc = _build_proj(M, N, 4096, 3, 8, outmod=4)
    wTb = np.ascontiguousarray(wT.astype(BF16))
    ins = [{"xT": np.ascontiguousarray(s), "wT": wTb,
            "b": np.ascontiguousarray(b.reshape(M, 1).astype(np.float32))}
           for s in xT_sh]
    res = bass_utils.run_bass_kernel_spmd(nc, ins, core_ids=list(range(NCORES)))
    if res.exec_time_ns:
        _EXEC_NS[0] += res.exec_time_ns
    return [r["out"] for r in res.results]


# ----------------------------------------------------------- host-side glue
def _grid_sample(img, grid):
    B, C, Hi, Wi = img.shape
    _, Hg, Wg, _ = grid.shape
    gx = (grid[..., 0] + 1.0) * 0.5 * (Wi - 1)
    gy = (grid[..., 1] + 1.0) * 0.5 * (Hi - 1)
    x0 = np.floor(gx); y0 = np.floor(gy)
    wx1 = gx - x0; wy1 = gy - y0
    flat = img.reshape(B, C, Hi * Wi)
    def corner(ix, iy, w):
        valid = (ix >= 0) & (ix <= Wi - 1) & (iy >= 0) & (iy <= Hi - 1)
        lin = (np.clip(iy, 0, Hi - 1) * Wi + np.clip(ix, 0, Wi - 1)).astype(np.int64)
        idx = np.broadcast_to(lin.reshape(B, 1, Hg * Wg), (B, C, Hg * Wg))
        v = np.take_along_axis(flat, idx, axis=2).reshape(B, C, Hg, Wg)
        return v * (w * valid)[:, None]
    return (corner(x0, y0, (1 - wx1) * (1 - wy1))
            + corner(x0 + 1, y0, wx1 * (1 - wy1))
            + corner(x0, y0 + 1, (1 - wx1) * wy1)
            + corner(x0 + 1, y0 + 1, wx1 * wy1))


def _ref_points(Hk, Wk, n):
    ry = ((np.arange(Hk, dtype=np.float32) + 0.5) / Hk) * 2 - 1
    rx = ((np.arange(Wk, dtype=np.float32) + 0.5) / Wk) * 2 - 1
    ref = np.stack(np.meshgrid(ry, rx, indexing='ij'), axis=-1)
    return np.broadcast_to(ref[None], (n, Hk, Wk, 2))


def kernel(x, wq, bq, wk, bk, wv, bv, wo, bo, off_dw_w, off_dw_b,
           off_ln_w, off_ln_b, off_pw_w, rpe_table, window_size):
    x = np.asarray(x, np.float32)
    ws = int(window_size)
    B, N, C = x.shape
    H = W = ws
    hc, gc, gh = C // N_HEAD, C // N_GROUP, N_HEAD // N_GROUP
    Bs = B // NCORES
    wq, wk, wv, wo = [np.asarray(a, np.float32) for a in (wq, wk, wv, wo)]

    # ---- q projection on device: q[d, (w,m)] = wq @ x^T + bq
    xT = np.ascontiguousarray(
        x.reshape(B, N, C).transpose(2, 0, 1).reshape(C, B * N).astype(BF16))
    shards = [np.ascontiguousarray(xT[:, i * Bs * N:(i + 1) * Bs * N])
              for i in range(NCORES)]
    qs = _run_proj(shards, wq.T, np.asarray(bq))
    q = np.concatenate(qs, axis=1).astype(np.float32)    # (C, B*N)
    q = q.reshape(C, B, N).transpose(1, 0, 2).reshape(B, C, H, W)

    # ---- offset network (host)
    q_off = q.reshape(B * N_GROUP, gc, H, W)
    Hk = Wk = (H + 2 - 3) // STRIDE + 1
    t = np.zeros((B * N_GROUP, gc, Hk, Wk), np.float32)
    qp = np.pad(q_off, ((0, 0), (0, 0), (1, 1), (1, 1)))
    dw = np.asarray(off_dw_w, np.float32)
    for ky in range(3):
        for kx in range(3):
            t += dw[None, :, 0, ky, kx, None, None] * \
                 qp[:, :, ky:ky + 2 * Hk:2, kx:kx + 2 * Wk:2]
    t += np.asarray(off_dw_b, np.float32)[None, :, None, None]
    tt = t.transpose(0, 2, 3, 1)
    mu = tt.mean(-1, keepdims=True)
    var = ((tt - mu) ** 2).mean(-1, keepdims=True)
    tt = (tt - mu) / np.sqrt(var + LN_EPS) * off_ln_w + off_ln_b
    from scipy.special import erf  # exact gelu
    tt = tt * 0.5 * (1.0 + erf(tt / np.sqrt(2.0)))
    offset = (tt.reshape(-1, gc) @ np.asarray(off_pw_w, np.float32).T) \
        .reshape(tt.shape[:3] + (2,))
    orange = np.array([1.0 / (Hk - 1), 1.0 / (Wk - 1)], np.float32)
    offset = np.tanh(offset) * orange * OFF_FACTOR
    n_sample = Hk * Wk
    pos = offset + _ref_points(Hk, Wk, B * N_GROUP)

    # ---- deformable gather (host) then k/v projections (device)
    xi = x.reshape(B, H, W, C).transpose(0, 3, 1, 2)
    xs = _grid_sample(xi.reshape(B * N_GROUP, gc, H, W), pos[..., ::-1])
    xs = xs.reshape(B, C, n_sample)
    xsT = np.ascontiguousarray(
        xs.transpose(1, 0, 2).reshape(C, B * n_sample).astype(BF16))
    shards = [np.ascontiguousarray(xsT[:, i * Bs * n_sample:(i + 1) * Bs * n_sample])
              for i in range(NCORES)]
    wkv = np.concatenate([wk.T, wv.T], axis=1)          # (256, 512)
    bkv = np.concatenate([np.asarray(bk), np.asarray(bv)])
    kvs = _run_proj(shards, wkv, bkv)
    kv = np.concatenate(kvs, axis=1).astype(np.float32)
    kv = kv.reshape(2 * C, B, n_sample).transpose(1, 0, 2)
    k, v = kv[:, :C], kv[:, C:]

    # ---- attention + rpe bias + softmax (host)
    qh = q.reshape(B * N_HEAD, hc, H * W)
    kh = k.reshape(B * N_HEAD, hc, n_sample)
    vh = v.reshape(B * N_HEAD, hc, n_sample)
    attn = np.matmul(qh.transpose(0, 2, 1), kh) * (hc ** -0.5)
    rpe = np.broadcast_to(np.asarray(rpe_table, np.float32)[None],
                          (B,) + rpe_table.shape)
    rpe = rpe.reshape(B * N_GROUP, gh, 2 * H - 1, 2 * W - 1)
    q_grid = _ref_points(H, W, B * N_GROUP).reshape(B * N_GROUP, H * W, 2)
    disp = (q_grid[:, :, None, :]
            - pos.reshape(B * N_GROUP, n_sample, 2)[:, None, :, :]) * 0.5
    bias = _grid_sample(rpe, disp[..., ::-1]).reshape(B * N_HEAD, H * W, n_sample)
    attn = attn + bias
    attn = attn - attn.max(axis=2, keepdims=True)
    e = np.exp(attn)
    attn = e / e.sum(axis=2, keepdims=True)
    out = np.matmul(vh, attn.transpose(0, 2, 1)).reshape(B, C, H * W)

    # ---- output projection on device
    outT = np.ascontiguousarray(
        out.transpose(1, 0, 2).reshape(C, B * N).astype(BF16))
    shards = [np.ascontiguousarray(outT[:, i * Bs * N:(i + 1) * Bs * N])
              for i in range(NCORES)]
    ys = _run_proj(shards, wo.T, np.asarray(bo))
    y = np.concatenate(ys, axis=1).astype(np.float32)
    y = y.reshape(C, B, N).transpose(1, 0, 2)
    return np.ascontiguousarray(y.reshape(B, C, H, W), dtype=np.float32)


# revision 20
# speedup vs baseline: 1.5412x; 1.1460x over previous
"""DAT Swin block kernel for 8 Trainium2 NeuronCores.

Device (Bass/Tile, data-parallel over the 4096-window batch, weights
replicated): the four dense 256x256 projections (q, k, v, o) — the dominant
FLOPs and all large-tensor HBM traffic — run in bf16 (1 PE cycle/row vs 4
for fp32, half the DMA bytes). Three launches (q; k+v fused; o), each
DMA-bandwidth-bound; tile sizes give 2-4KB per-partition DMA lines and
matmuls are split into 512-col halves so no PSUM bank boundary is crossed.
Host (numpy): the data-dependent glue that is layout-hostile on device
(offset network with cross-channel LN, bilinear gathers, 64x16 softmax).
"""
import sys, json

sys.path.insert(0, '/opt/trn_rl_repo')
import numpy as np
import concourse.bass as bass
import concourse.mybir as mybir
from concourse.tile import TileContext
from concourse import bass_utils

N_HEAD, N_GROUP, STRIDE, OFF_FACTOR, LN_EPS = 8, 4, 2, 2.0, 1e-5
NCORES = 8
BF16 = mybir.dt.np(mybir.dt.bfloat16)

# ---------------------------------------------------------------- BIR fix:
# this walrus build allows at most ONE sync wait per instruction; split
# extras onto same-engine NoOps inserted just before the instruction.
def _split_waits(m):
    ctr = [0]
    for f in m["functions"]:
        for b in f["blocks"]:
            out = []
            for i in b["instructions"]:
                si = i.get("sync_info")
                waits = (si or {}).get("on_wait") or []
                if len(waits) > 1:
                    for w in waits[:-1]:
                        ctr[0] += 1
                        out.append({"name": f"nopw-{ctr[0]}", "opcode": "NoOp",
                                    "engine": i.get("engine"), "ins": [], "outs": [],
                                    "sync_info": {"on_update": [], "on_wait": [w]}})
                    si["on_wait"] = waits[-1:]
                out.append(i)
            b["instructions"] = out
    return m


def _patch(nc):
    orig = nc.to_json_bytes
    nc.to_json_bytes = lambda: json.dumps(_split_waits(json.loads(orig()))).encode()
    return nc


# ------------------------------------------------- device projection kernel
# out[M, N] = wT.T @ xT + b   (wT: (256, M) bf16, xT: (256, N) bf16, b: (M,))
_NC_CACHE = {}


def _build_proj(M, N, nchunk=1024, iobufs=3, psbufs=2, storep=("act", "sp")):
    """storep: store-DMA issue engines, cycled per store. Loads issue from
    SP; moving stores onto other sequencers (ACT HWDGE / Pool SWDGE) keeps
    dependency-blocked stores from stalling input prefetch and lets in/out
    transfers overlap."""
    key = (M, N, nchunk, iobufs, psbufs, tuple(storep))
    if key in _NC_CACHE:
        return _NC_CACHE[key]
    nc = bass.Bass()
    f32 = mybir.dt.float32
    bf16 = mybir.dt.bfloat16
    xT = nc.dram_tensor("xT", (256, N), bf16, kind="ExternalInput")
    wT = nc.dram_tensor("wT", (256, M), bf16, kind="ExternalInput")
    bb = nc.dram_tensor("b", (M, 1), f32, kind="ExternalInput")
    out = nc.dram_tensor("out", (M, N), bf16, kind="ExternalOutput")
    n_mt = M // 128
    with TileContext(nc) as tc:
        with tc.tile_pool(name="w", bufs=1) as wp, \
             tc.tile_pool(name="io", bufs=iobufs) as iop, \
             tc.tile_pool(name="ps", bufs=psbufs, space="PSUM") as pp:
            wtiles = {}
            btiles = {}
            for kt in range(2):
                t = wp.tile([128, M], bf16, tag=f"w{kt}")
                nc.sync.dma_start(t[:, :], wT[kt * 128:(kt + 1) * 128, :])
                wtiles[kt] = t
            for mt in range(n_mt):
                bt = wp.tile([128, 1], f32, tag=f"b{mt}")
                nc.sync.dma_start(bt[:, :], bb[mt * 128:(mt + 1) * 128, :])
                btiles[mt] = bt
            for ci, n0 in enumerate(range(0, N, nchunk)):
                rhs = []
                for kt in range(2):
                    t = iop.tile([128, nchunk], bf16, tag=f"x{kt}")
                    nc.sync.dma_start(t[:, :], xT[kt * 128:(kt + 1) * 128,
                                                   n0:n0 + nchunk])
                    rhs.append(t)
                for mt in range(n_mt):
                    ot = iop.tile([128, nchunk], bf16, tag=f"o{mt % 2}")
                    # PSUM bank is 512 fp32 wide; matmul output cannot cross
                    # a bank boundary, so compute in 512-wide halves.
                    for h0 in range(0, nchunk, 512):
                        ps = pp.tile([128, 512], f32, tag="ps")
                        for kt in range(2):
                            nc.tensor.matmul(ps[:, :],
                                             wtiles[kt][:, mt * 128:(mt + 1) * 128],
                                             rhs[kt][:, h0:h0 + 512],
                                             start=(kt == 0), stop=(kt == 1))
                        nc.any.tensor_scalar_add(ot[:, h0:h0 + 512], ps[:, :],
                                                 btiles[mt][:, :])
                    idx = ci * n_mt + mt
                    oeng = {"sp": nc.sync, "act": nc.scalar,
                            "pool": nc.gpsimd}[storep[idx % len(storep)]]
                    oeng.dma_start(out[mt * 128:(mt + 1) * 128,
                                       n0:n0 + nchunk], ot[:, :])
    _NC_CACHE[key] = _patch(nc)
    return _NC_CACHE[key]


_EXEC_NS = [0]


def _run_proj(xT_sh, wT, b):
    """xT_sh: list of NCORES bf16 arrays (256, N). Returns list of (M, N)."""
    M = wT.shape[1]
    N = xT_sh[0].shape[1]
    # pool sizes and store-issue pattern tuned via CoreSim sweep (sweep.py)
    if M == 512:
        nc = _build_proj(M, N, 1024, 6, 4, storep=("act", "sp"))
    else:
        nc = _build_proj(M, N, 1024, 6, 8, storep=("pool",))
    wTb = np.ascontiguousarray(wT.astype(BF16))
    ins = [{"xT": np.ascontiguousarray(s), "wT": wTb,
            "b": np.ascontiguousarray(b.reshape(M, 1).astype(np.float32))}
           for s in xT_sh]
    res = bass_utils.run_bass_kernel_spmd(nc, ins, core_ids=list(range(NCORES)))
    if res.exec_time_ns:
        _EXEC_NS[0] += res.exec_time_ns
    return [r["out"] for r in res.results]


# ----------------------------------------------------------- host-side glue
def _grid_sample(img, grid):
    B, C, Hi, Wi = img.shape
    _, Hg, Wg, _ = grid.shape
    gx = (grid[..., 0] + 1.0) * 0.5 * (Wi - 1)
    gy = (grid[..., 1] + 1.0) * 0.5 * (Hi - 1)
    x0 = np.floor(gx); y0 = np.floor(gy)
    wx1 = gx - x0; wy1 = gy - y0
    flat = img.reshape(B, C, Hi * Wi)
    def corner(ix, iy, w):
        valid = (ix >= 0) & (ix <= Wi - 1) & (iy >= 0) & (iy <= Hi - 1)
        lin = (np.clip(iy, 0, Hi - 1) * Wi + np.clip(ix, 0, Wi - 1)).astype(np.int64)
        idx = np.broadcast_to(lin.reshape(B, 1, Hg * Wg), (B, C, Hg * Wg))
        v = np.take_along_axis(flat, idx, axis=2).reshape(B, C, Hg, Wg)
        return v * (w * valid)[:, None]
    return (corner(x0, y0, (1 - wx1) * (1 - wy1))
            + corner(x0 + 1, y0, wx1 * (1 - wy1))
            + corner(x0, y0 + 1, (1 - wx1) * wy1)
            + corner(x0 + 1, y0 + 1, wx1 * wy1))


def _ref_points(Hk, Wk, n):
    ry = ((np.arange(Hk, dtype=np.float32) + 0.5) / Hk) * 2 - 1
    rx = ((np.arange(Wk, dtype=np.float32) + 0.5) / Wk) * 2 - 1
    ref = np.stack(np.meshgrid(ry, rx, indexing='ij'), axis=-1)
    return np.broadcast_to(ref[None], (n, Hk, Wk, 2))


def kernel(x, wq, bq, wk, bk, wv, bv, wo, bo, off_dw_w, off_dw_b,
           off_ln_w, off_ln_b, off_pw_w, rpe_table, window_size):
    x = np.asarray(x, np.float32)
    ws = int(window_size)
    B, N, C = x.shape
    H = W = ws
    hc, gc, gh = C // N_HEAD, C // N_GROUP, N_HEAD // N_GROUP
    Bs = B // NCORES
    wq, wk, wv, wo = [np.asarray(a, np.float32) for a in (wq, wk, wv, wo)]

    # ---- q projection on device: q[d, (w,m)] = wq @ x^T + bq
    xT = np.ascontiguousarray(
        x.reshape(B, N, C).transpose(2, 0, 1).reshape(C, B * N).astype(BF16))
    shards = [np.ascontiguousarray(xT[:, i * Bs * N:(i + 1) * Bs * N])
              for i in range(NCORES)]
    qs = _run_proj(shards, wq.T, np.asarray(bq))
    q = np.concatenate(qs, axis=1).astype(np.float32)    # (C, B*N)
    q = q.reshape(C, B, N).transpose(1, 0, 2).reshape(B, C, H, W)

    # ---- offset network (host)
    q_off = q.reshape(B * N_GROUP, gc, H, W)
    Hk = Wk = (H + 2 - 3) // STRIDE + 1
    t = np.zeros((B * N_GROUP, gc, Hk, Wk), np.float32)
    qp = np.pad(q_off, ((0, 0), (0, 0), (1, 1), (1, 1)))
    dw = np.asarray(off_dw_w, np.float32)
    for ky in range(3):
        for kx in range(3):
            t += dw[None, :, 0, ky, kx, None, None] * \
                 qp[:, :, ky:ky + 2 * Hk:2, kx:kx + 2 * Wk:2]
    t += np.asarray(off_dw_b, np.float32)[None, :, None, None]
    tt = t.transpose(0, 2, 3, 1)
    mu = tt.mean(-1, keepdims=True)
    var = ((tt - mu) ** 2).mean(-1, keepdims=True)
    tt = (tt - mu) / np.sqrt(var + LN_EPS) * off_ln_w + off_ln_b
    from scipy.special import erf  # exact gelu
    tt = tt * 0.5 * (1.0 + erf(tt / np.sqrt(2.0)))
    offset = (tt.reshape(-1, gc) @ np.asarray(off_pw_w, np.float32).T) \
        .reshape(tt.shape[:3] + (2,))
    orange = np.array([1.0 / (Hk - 1), 1.0 / (Wk - 1)], np.float32)
    offset = np.tanh(offset) * orange * OFF_FACTOR
    n_sample = Hk * Wk
    pos = offset + _ref_points(Hk, Wk, B * N_GROUP)

    # ---- deformable gather (host) then k/v projections (device)
    xi = x.reshape(B, H, W, C).transpose(0, 3, 1, 2)
    xs = _grid_sample(xi.reshape(B * N_GROUP, gc, H, W), pos[..., ::-1])
    xs = xs.reshape(B, C, n_sample)
    xsT = np.ascontiguousarray(
        xs.transpose(1, 0, 2).reshape(C, B * n_sample).astype(BF16))
    shards = [np.ascontiguousarray(xsT[:, i * Bs * n_sample:(i + 1) * Bs * n_sample])
              for i in range(NCORES)]
    wkv = np.concatenate([wk.T, wv.T], axis=1)          # (256, 512)
    bkv = np.concatenate([np.asarray(bk), np.asarray(bv)])
    kvs = _run_proj(shards, wkv, bkv)
    kv = np.concatenate(kvs, axis=1).astype(np.float32)
    kv = kv.reshape(2 * C, B, n_sample).transpose(1, 0, 2)
    k, v = kv[:, :C], kv[:, C:]

    # ---- attention + rpe bias + softmax (host)
    qh = q.reshape(B * N_HEAD, hc, H * W)
    kh = k.reshape(B * N_HEAD, hc, n_sample)
    vh = v.reshape(B * N_HEAD, hc, n_sample)
    attn = np.matmul(qh.transpose(0, 2, 1), kh) * (hc ** -0.5)
    rpe = np.broadcast_to(np.asarray(rpe_table, np.float32)[None],
                          (B,) + rpe_table.shape)
    rpe = rpe.reshape(B * N_GROUP, gh, 2 * H - 1, 2 * W - 1)
    q_grid = _ref_points(H, W, B * N_GROUP).reshape(B * N_GROUP, H * W, 2)
    disp = (q_grid[:, :, None, :]
            - pos.reshape(B * N_GROUP, n_sample, 2)[:, None, :, :]) * 0.5
    bias = _grid_sample(rpe, disp[..., ::-1]).reshape(B * N_HEAD, H * W, n_sample)
    attn = attn + bias
    attn = attn - attn.max(axis=2, keepdims=True)
    e = np.exp(attn)
    attn = e / e.sum(axis=2, keepdims=True)
    out = np.matmul(vh, attn.transpose(0, 2, 1)).reshape(B, C, H * W)

    # ---- output projection on device
    outT = np.ascontiguousarray(
        out.transpose(1, 0, 2).reshape(C, B * N).astype(BF16))
    shards = [np.ascontiguousarray(outT[:, i * Bs * N:(i + 1) * Bs * N])
              for i in range(NCORES)]
    ys = _run_proj(shards, wo.T, np.asarray(bo))
    y = np.concatenate(ys, axis=1).astype(np.float32)
    y = y.reshape(C, B, N).transpose(1, 0, 2)
    return np.ascontiguousarray(y.reshape(B, C, H, W), dtype=np.float32)
